# revision 16
# baseline (speedup 1.0000x reference)
"""nn_CrossAttention kernel for 8 Trainium2 NeuronCores.

Sharding: data-parallel over batch B=8, one batch element per core, no
collectives. Per-core layout keeps activations transposed ([feature,
token]); weights load as the stationary matmul operand.

Attention uses keys-on-partition S^T tiles (bf16 matmuls) so the PV
contraction runs over keys. exp() outputs fp8e4m3 pair-tiles
([128, 2, NT]) feeding DoubleRow fp8 matmuls: PV and the softmax
denominator (ones-row) contract 256 keys at 0.5 cycles/row — 4x the
bf16 rate. V is pre-scaled by 4 and the normalized per-head outputs by
16 (stored fp8) to stay in fp8e4m3's normal range; output-projection
weights are stored as 8x-scaled fp8 DoubleRow head-pair tiles and the
512x net scale is divided out in the final scalar_tensor_tensor.

The per-head projection work (phase C) is software-pipelined into the
attention loop so the Activation engine's exp stream (the critical
~133us) is never starved while the PE fills its slack with projections.
"""
import sys

sys.path.insert(0, "/opt/trn_rl_repo")

import numpy as np
import ml_dtypes

import concourse.bass as bass
import concourse.tile as tile
from concourse import bacc, mybir, bass2jax

F32 = mybir.dt.float32
BF16 = mybir.dt.bfloat16
F8 = mybir.dt.float8e4
EXP = mybir.ActivationFunctionType.Exp
COPY = mybir.ActivationFunctionType.Copy
DR = mybir.MatmulPerfMode.DoubleRow
ADD = mybir.AluOpType.add
MULT = mybir.AluOpType.mult

N_CORES = 8
H, D = 8, 64          # heads, head_dim
D2 = 2 * D            # 128
NT = 1024             # tokens
C = 512               # model dim
KB = 8                # key blocks of 128
SCALE = D ** -0.5
V_SCALE = 4.0         # v stored x4 (folded into w*v weights)
O_SCALE = 16.0        # normalize-mul extra scale -> o stored x64 in fp8
WP_SCALE = 8.0        # wp1/wp2 stored x8 in fp8
E_DESCALE = 1.0 / (V_SCALE * O_SCALE * WP_SCALE)


def _build(nc):
    dram = {}
    def din(name, shape, dt):
        dram[name] = nc.dram_tensor(name, shape, dt, kind="ExternalInput").ap()
    din("xT", [84, NT], BF16)
    din("yT", [50, NT], BF16)
    din("W1", [84, C], BF16)
    din("W2", [50, C], BF16)
    for n in ("w1k", "w2k", "w1v", "w2v"):
        din(n, [C, 512], BF16)
    din("w1q", [C, 1024], BF16)
    din("w2q", [C, 1024], BF16)
    din("wk2", [D, D2], BF16)
    for i in range(4):
        din(f"wp1dr{i}", [D2, 2, C], F8)
        din(f"wp2dr{i}", [D, 2, C], F8)
    din("bp1", [C], F32)
    din("bp2", [C], F32)
    outT = nc.dram_tensor("outT", [2 * C, NT], F32, kind="ExternalOutput").ap()

    with tile.TileContext(nc) as tc:
        _body(tc, nc, dram, outT)
    return dram, outT


def _body(tc, nc, dram, outT):
    from contextlib import ExitStack
    ctx = ExitStack()
    with ctx:
        wts = ctx.enter_context(tc.tile_pool(name="wts", bufs=1))
        acts = ctx.enter_context(tc.tile_pool(name="acts", bufs=1))
        big = ctx.enter_context(tc.tile_pool(name="big", bufs=2, space="PSUM"))
        psO = ctx.enter_context(tc.tile_pool(name="psO", bufs=2, space="PSUM"))
        k2pool = ctx.enter_context(tc.tile_pool(name="k2pool", bufs=4))
        ptpool = ctx.enter_context(tc.tile_pool(name="ptpool", bufs=3))
        rpool = ctx.enter_context(tc.tile_pool(name="rpool", bufs=2))
        rbpool = ctx.enter_context(tc.tile_pool(name="rbpool", bufs=2))
        outp = ctx.enter_context(tc.tile_pool(name="outp", bufs=2))

        def load(pool, name, shape, dt, src_ap=None):
            t = pool.tile(shape, dt, tag=name, name=name)
            nc.sync.dma_start(out=t, in_=dram[name] if src_ap is None else src_ap)
            return t

        # ---- inputs first so phase A starts immediately ----
        xts = load(wts, "xT", [84, NT], BF16)
        yts = load(wts, "yT", [50, NT], BF16)
        w1 = load(wts, "W1", [84, C], BF16)
        w2 = load(wts, "W2", [50, C], BF16)

        # persistent activations
        xcb = [acts.tile([128, NT], BF16, tag=f"xcb{j}", name=f"xcb{j}") for j in range(4)]
        ycb = [acts.tile([128, NT], BF16, tag=f"ycb{j}", name=f"ycb{j}") for j in range(4)]
        knew = [acts.tile([D2, NT], BF16, tag=f"kn{h}", name=f"kn{h}") for h in range(H)]
        q1p = [acts.tile([D2, NT], BF16, tag=f"q1p{h}", name=f"q1p{h}") for h in range(H)]
        q2p = [acts.tile([D2, NT], BF16, tag=f"q2p{h}", name=f"q2p{h}") for h in range(H)]
        # fp8 v pair tiles: [128, H, 2(kb-slot), 144]; col 128 = ones row
        vaugp = [acts.tile([128, H, 2, 144], F8, tag=f"va{p}", name=f"va{p}") for p in range(4)]
        # fp8 DR pair tiles for output projection operands
        o1n = [acts.tile([D2, 2, NT], F8, tag=f"o1n{i}", name=f"o1n{i}") for i in range(4)]
        o2n = [acts.tile([D, 2, NT], F8, tag=f"o2n{i}", name=f"o2n{i}") for i in range(4)]
        resqb = [acts.tile([128, NT], BF16, tag=f"rq{j}", name=f"rq{j}") for j in range(8)]
        onesdr = wts.tile([128, 2, 16], F8, tag="onesdr", name="onesdr")
        nc.vector.memset(onesdr, 1.0)
        for p in range(4):
            nc.vector.memset(vaugp[p][:, :, :, 128:129], 1.0)

        # ---- phase A: xcb = (W1^T @ xT) bf16, ycb = (W2^T @ yT) ----
        for (w, src, dstb, on_act) in ((w1, xts, xcb, True), (w2, yts, ycb, False)):
            kdim = w.shape[0]
            for j in range(4):
                ps = big.tile([128, NT], F32, tag="ps", name="psA")
                for nb in range(2):
                    nc.tensor.matmul(ps[:, nb * 512:(nb + 1) * 512],
                                     w[0:kdim, j * 128:(j + 1) * 128],
                                     src[0:kdim, nb * 512:(nb + 1) * 512],
                                     start=True, stop=True)
                if on_act:
                    nc.scalar.activation(dstb[j], ps, COPY)
                else:
                    nc.vector.tensor_copy(dstb[j], ps)

        # ---- remaining weight loads (in exact use order) ----
        wk_tiles = {}
        for wk in ("w1k", "w2k"):
            wk_tiles[wk] = [load(wts, f"{wk}_{k}", [128, 512], BF16,
                                 dram[wk][k * 128:(k + 1) * 128, :]) for k in range(4)]
        wq_tiles = {"w1q": [load(wts, f"w1q_{k}", [128, 1024], BF16,
                                 dram["w1q"][k * 128:(k + 1) * 128, :])
                            for k in range(4)]}
        wv_tiles = {}
        for wv in ("w1v", "w2v"):
            wv_tiles[wv] = [load(wts, f"{wv}_{k}", [128, 512], BF16,
                                 dram[wv][k * 128:(k + 1) * 128, :]) for k in range(4)]
        wq_tiles["w2q"] = [load(wts, f"w2q_{k}", [128, 1024], BF16,
                               dram["w2q"][k * 128:(k + 1) * 128, :])
                           for k in range(4)]
        wk2 = wts.tile([D2, D2], BF16, tag="wk2", name="wk2")
        nc.sync.dma_start(out=wk2[D:D2, :], in_=dram["wk2"])
        wp1 = [load(wts, f"wp1dr{i}", [D2, 2, C], F8) for i in range(4)]
        wp2 = [load(wts, f"wp2dr{i}", [D, 2, C], F8) for i in range(4)]
        bp1 = wts.tile([128, 4], F32, tag="bp1", name="bp1")
        nc.sync.dma_start(out=bp1, in_=dram["bp1"].rearrange("(j p) -> p j", j=4))
        bp2 = wts.tile([128, 4], F32, tag="bp2", name="bp2")
        nc.sync.dma_start(out=bp2, in_=dram["bp2"].rearrange("(j p) -> p j", j=4))

        # ---- work chunk generators (each chunk ~1-2us of PE) ----
        def chunk_v(kb, side):
            # v projection for key block kb, one side -> vaugp slot
            wvt = wv_tiles["w1v" if side == 0 else "w2v"]
            srcb = xcb if side == 0 else ycb
            lo = 0 if side == 0 else D
            ps = big.tile([128, NT], F32, tag="ps", name="psB")
            for k in range(4):
                nc.tensor.matmul(ps[:, 0:512], srcb[k][:, kb * 128:(kb + 1) * 128],
                                 wvt[k], start=(k == 0), stop=(k == 3))
            dst = vaugp[kb // 2][:, :, kb % 2, lo:lo + D]
            src = ps[:, 0:512].rearrange("p (h d) -> p h d", h=H)
            if side == 0:
                nc.vector.tensor_copy(dst, src)
            else:
                nc.gpsimd.tensor_copy(dst, src)

        def chunk_k(pr, side):
            # k projection for head pair (2pr, 2pr+1), one side
            wkt = wk_tiles["w1k" if side == 0 else "w2k"]
            srcb = xcb if side == 0 else ycb
            lo = 0 if side == 0 else D
            ps = big.tile([128, NT], F32, tag="ps", name="psCk")
            for nb in range(2):
                for k in range(4):
                    nc.tensor.matmul(ps[:, nb * 512:(nb + 1) * 512],
                                     wkt[k][:, pr * 128:(pr + 1) * 128],
                                     srcb[k][:, nb * 512:(nb + 1) * 512],
                                     start=(k == 0), stop=(k == 3))
            if side == 0:
                nc.vector.tensor_copy(knew[2 * pr][lo:lo + D, :], ps[0:D, :])
                nc.gpsimd.tensor_copy(knew[2 * pr + 1][lo:lo + D, :], ps[D:128, :])
            else:
                nc.gpsimd.tensor_copy(knew[2 * pr][lo:lo + D, :], ps[0:D, :])
                nc.vector.tensor_copy(knew[2 * pr + 1][lo:lo + D, :], ps[D:128, :])

        def chunk_q(h, side):
            # folded q projection for head h, one side (side0 -> q1p, side1 -> q2p)
            wqt = wq_tiles["w1q" if side == 0 else "w2q"]
            srcb = xcb if side == 0 else ycb
            dst = q1p if side == 0 else q2p
            ps = big.tile([128, NT], F32, tag="ps", name="psCq")
            for nb in range(2):
                for k in range(4):
                    nc.tensor.matmul(ps[:, nb * 512:(nb + 1) * 512],
                                     wqt[k][:, h * 128:(h + 1) * 128],
                                     srcb[k][:, nb * 512:(nb + 1) * 512],
                                     start=(k == 0), stop=(k == 3))
            if side == 0:
                nc.vector.tensor_copy(dst[h], ps)
                # residual + q1r + bias, precomputed for phase E
                rsrc = xcb[h] if h < 4 else ycb[h - 4]
                bias = bp1[:, h:h + 1] if h < 4 else bp2[:, h - 4:h - 3]
                nc.vector.scalar_tensor_tensor(resqb[h], rsrc, bias, q1p[h], ADD, ADD)
            else:
                nc.gpsimd.tensor_copy(dst[h], ps)

        k2pt = {}
        def chunk_k2p(h):
            # k2p = Wk2^T @ k2 for head h (base-64 operands)
            ps = big.tile([128, NT], F32, tag="ps", name="psK2")
            for nb in range(2):
                nc.tensor.matmul(ps[:, nb * 512:(nb + 1) * 512],
                                 wk2[D:D2, :],
                                 knew[h][D:D2, nb * 512:(nb + 1) * 512],
                                 start=True, stop=True)
            t = k2pool.tile([D2, NT], BF16, tag="k2p", name=f"k2p{h}")
            nc.vector.tensor_copy(t, ps)
            k2pt[h] = t

        def c_chunks(pr):
            yield lambda: chunk_k(pr, 0)
            yield lambda: chunk_k(pr, 1)
            yield lambda: chunk_q(2 * pr, 0)
            yield lambda: chunk_q(2 * pr + 1, 0)
            yield lambda: chunk_q(2 * pr, 1)
            yield lambda: chunk_q(2 * pr + 1, 1)
            yield lambda: chunk_k2p(2 * pr)
            yield lambda: chunk_k2p(2 * pr + 1)

        # ---- prologue: minimal work before the exp stream starts ----
        chunk_k(0, 0)
        chunk_k(0, 1)
        chunk_q(0, 0)
        for kb in range(4):
            for s in range(2):
                chunk_v(kb, s)

        # chunk schedule: two projection chunks at each branch end, placed
        # >= half a head before their consumer (PE burst hides in the Act
        # runway of already-queued exps)
        K, Q, P2 = chunk_k, chunk_q, chunk_k2p
        pops = {
            (0, 0): [lambda: Q(1, 0), lambda: Q(1, 1)],
            (0, 1): [lambda: P2(1), lambda: Q(2, 0)],
            (1, 0): [lambda: K(1, 0), lambda: K(1, 1)],
            (1, 1): [lambda: Q(2, 1), lambda: P2(2)],
            (2, 0): [lambda: Q(3, 0), lambda: Q(3, 1)],
            (2, 1): [lambda: P2(3), lambda: K(2, 0)],
            (3, 0): [lambda: K(2, 1), lambda: Q(4, 0)],
            (3, 1): [lambda: Q(4, 1), lambda: P2(4)],
            (4, 0): [lambda: Q(5, 0), lambda: Q(5, 1)],
            (4, 1): [lambda: P2(5), lambda: K(3, 0)],
            (5, 0): [lambda: K(3, 1), lambda: Q(6, 0)],
            (5, 1): [lambda: Q(6, 1), lambda: P2(6)],
            (6, 0): [lambda: Q(7, 0), lambda: Q(7, 1)],
            (6, 1): [lambda: P2(7)],
        }

        # ---- attention: PVs deferred one pair behind the exp stream so
        # accumulator-psum WARs never block it; head 0 br1 fully defers and
        # absorbs the remaining v fills ----
        def attention_head(h):
            defer = (h == 0)
            ops1 = psO.tile([128, NT], F32, tag="psO", name=f"ops1_{h}")
            ops2 = psO.tile([128, NT], F32, tag="psO", name=f"ops2_{h}")
            for br in range(2):
                lhs = knew[h] if br == 0 else k2pt[h]
                qin = q1p[h] if br == 0 else q2p[h]
                pend = []
                for pr in range(4):
                    ptp = ptpool.tile([128, 2, NT], F8, tag="pt", name="pt")
                    pend.append((pr, ptp))
                    for sl in range(2):
                        kb = 2 * pr + sl
                        sps = big.tile([128, NT], F32, tag="ps", name="psS")
                        for nb in range(2):
                            nc.tensor.matmul(sps[:, nb * 512:(nb + 1) * 512],
                                             lhs[:, kb * 128:(kb + 1) * 128],
                                             qin[:, nb * 512:(nb + 1) * 512],
                                             start=True, stop=True)
                        nc.scalar.activation(ptp[:, sl, :], sps, EXP, scale=SCALE)
                        if defer and br == 0:
                            chunk_v(4 + kb // 2, kb % 2)  # v kb4..7
                    if defer and br == 0:
                        continue  # PVs deferred until all v ready
                    if len(pend) > 1:
                        p0, t0 = pend.pop(0)
                        self_pv(br, p0, t0, h, ops1, ops2)
                if defer and br == 0:
                    chunk_q(0, 1)      # q2p[0]: needed at br2 start
                    chunk_k2p(0)       # k2pt[0]: needed at br2 start
                for p0, t0 in pend:
                    self_pv(br, p0, t0, h, ops1, ops2)
                for f in pops.pop((h, br), []):
                    f()
                # normalize
                i, sl8 = h // 2, h % 2
                rr = rpool.tile([1, NT], F32, tag="rr", name="rr")
                rrb = rbpool.tile([128, NT], F32, tag="rrb", name="rrb")
                if br == 0:
                    nc.vector.reciprocal(rr, ops2[0:1, :])
                    nc.gpsimd.partition_broadcast(rrb, rr)
                    nc.vector.scalar_tensor_tensor(
                        o1n[i][:, sl8, :], ops1[:], O_SCALE, rrb, MULT, MULT)
                else:
                    nc.vector.reciprocal(rr, ops2[D:D + 1, :])
                    nc.gpsimd.partition_broadcast(rrb[0:D, :], rr)
                    nc.vector.scalar_tensor_tensor(
                        o2n[i][:, sl8, :], ops2[0:D, :], O_SCALE, rrb[0:D, :],
                        MULT, MULT)

        def self_pv(br, pr, ptp, h, ops1, ops2):
            for nb in range(2):
                sl_ = slice(nb * 512, (nb + 1) * 512)
                if br == 0:
                    nc.tensor.matmul(ops1[:, sl_], vaugp[pr][:, h, :, 0:D2],
                                     ptp[:, :, sl_], start=(pr == 0),
                                     stop=(pr == 3), perf_mode=DR)
                    nc.tensor.matmul(ops2[0:1, sl_], onesdr[:, :, 0:1],
                                     ptp[:, :, sl_], start=(pr == 0),
                                     stop=(pr == 3), perf_mode=DR)
                else:
                    nc.tensor.matmul(ops2[0:D + 1, sl_],
                                     vaugp[pr][:, h, :, D:D2 + 1],
                                     ptp[:, :, sl_], start=(pr == 0),
                                     stop=(pr == 3), perf_mode=DR,
                                     skip_group_check=True)

        for h in range(H):
            attention_head(h)
        assert not pops, f"{len(pops)} chunk slots unconsumed"

        # ---- phase E: output projections (fp8 DR) + residuals ----
        for (wp, on, rqoff, rowoff) in ((wp1, o1n, 0, 0), (wp2, o2n, 4, C)):
            for j in range(4):
                zps = big.tile([128, NT], F32, tag="ps", name="psE")
                for nb in range(2):
                    sl_ = slice(nb * 512, (nb + 1) * 512)
                    for i in range(4):
                        nc.tensor.matmul(zps[:, sl_],
                                         wp[i][:, :, j * 128:(j + 1) * 128],
                                         on[i][:, :, sl_],
                                         start=(i == 0), stop=(i == 3),
                                         perf_mode=DR)
                of = outp.tile([128, NT], F32, tag="of", name="of")
                eng = nc.vector if j % 2 == 0 else nc.gpsimd
                eng.scalar_tensor_tensor(of, zps, E_DESCALE,
                                         resqb[rqoff + j], MULT, ADD)
                nc.sync.dma_start(
                    out=outT[rowoff + j * 128:rowoff + (j + 1) * 128, :], in_=of)


class _Runner:
    def __init__(self):
        import jax
        from jax.sharding import Mesh, PartitionSpec
        from jax.experimental.shard_map import shard_map

        nc = bacc.Bacc("TRN2", target_bir_lowering=False, debug=False,
                       num_devices=N_CORES)
        _build(nc)
        nc.compile()
        self.nc = nc

        bass2jax.install_neuronx_cc_hook()
        part_name = nc.partition_id_tensor.name if nc.partition_id_tensor else None
        in_names, out_names, out_avals, self.zero_shapes = [], [], [], []
        for alloc in nc.m.functions[0].allocations:
            if not isinstance(alloc, mybir.MemoryLocationSet):
                continue
            name = alloc.memorylocations[0].name
            if alloc.kind == "ExternalInput":
                if name != part_name:
                    in_names.append(name)
            elif alloc.kind == "ExternalOutput":
                out_names.append(name)
                shape = tuple(alloc.tensor_shape)
                dtype = mybir.dt.np(alloc.dtype)
                out_avals.append(jax.core.ShapedArray(shape, dtype))
                self.zero_shapes.append((shape, dtype))
        self.in_names, self.out_names, self.out_avals = in_names, out_names, out_avals
        n_params, n_outs = len(in_names), len(out_avals)
        all_names = in_names + out_names + ([part_name] if part_name else [])

        def _bodyfn(*args):
            operands = list(args)
            if part_name:
                operands.append(bass2jax.partition_id_tensor())
            outs = bass2jax._bass_exec_p.bind(
                *operands, out_avals=tuple(out_avals), in_names=tuple(all_names),
                out_names=tuple(out_names), lowering_input_output_aliases=(),
                sim_require_finite=True, sim_require_nnan=True, nc=nc)
            return tuple(outs)

        devices = jax.devices()[:N_CORES]
        mesh = Mesh(np.asarray(devices), ("core",))
        self._fn = jax.jit(
            shard_map(_bodyfn, mesh=mesh,
                      in_specs=(PartitionSpec("core"),) * (n_params + n_outs),
                      out_specs=(PartitionSpec("core"),) * n_outs,
                      check_rep=False),
            donate_argnums=tuple(range(n_params, n_params + n_outs)),
            keep_unused=True)
        self._jax = jax

    def __call__(self, in_maps):
        concat_in = [np.concatenate([m[n] for m in in_maps], axis=0)
                     for n in self.in_names]
        zeros = [np.zeros((N_CORES * s[0], *s[1:]), d) for s, d in self.zero_shapes]
        outs = self._fn(*concat_in, *zeros)
        self._jax.block_until_ready(outs)
        return [
            {n: np.asarray(outs[i]).reshape(N_CORES, *self.out_avals[i].shape)[c]
             for i, n in enumerate(self.out_names)}
            for c in range(N_CORES)
        ]


_RUNNER = None


def _get_runner():
    global _RUNNER
    if _RUNNER is None:
        _RUNNER = _Runner()
    return _RUNNER


def _prep_in_maps(inputs):
    f32 = np.float32
    bf = ml_dtypes.bfloat16
    f8 = ml_dtypes.float8_e4m3
    x = np.asarray(inputs["x"], f32)
    y = np.asarray(inputs["y"], f32)
    Wqkv1 = np.asarray(inputs["Wqkv1"], np.float64)
    Wqkv2 = np.asarray(inputs["Wqkv2"], np.float64)
    Wq1 = np.asarray(inputs["Wq1"], np.float64)
    Wq2 = np.asarray(inputs["Wq2"], np.float64)
    w1q = np.zeros((C, 1024), np.float64)
    w2q = np.zeros((C, 1024), np.float64)
    for h in range(H):
        w1q[:, h * D2:(h + 1) * D2] = Wqkv1[:, h * D:(h + 1) * D] @ Wq1
        w2q[:, h * D2:(h + 1) * D2] = Wqkv2[:, h * D:(h + 1) * D] @ Wq2
    Wp1 = np.asarray(inputs["Wp1"], f32)
    Wp2 = np.asarray(inputs["Wp2"], f32)
    shared = {
        "W1": np.ascontiguousarray(inputs["W1"]).astype(bf),
        "W2": np.ascontiguousarray(inputs["W2"]).astype(bf),
        "w1k": Wqkv1[:, 512:1024].astype(bf),
        "w2k": Wqkv2[:, 512:1024].astype(bf),
        "w1v": (Wqkv1[:, 1024:1536] * V_SCALE).astype(bf),
        "w2v": (Wqkv2[:, 1024:1536] * V_SCALE).astype(bf),
        "w1q": w1q.astype(bf),
        "w2q": w2q.astype(bf),
        "wk2": np.asarray(inputs["Wk2"]).astype(bf),
        "bp1": np.ascontiguousarray(inputs["bp1"], f32),
        "bp2": np.ascontiguousarray(inputs["bp2"], f32),
    }
    for i in range(4):
        t1 = np.empty((D2, 2, C), f32)
        t2 = np.empty((D, 2, C), f32)
        for j in range(2):
            h = 2 * i + j
            t1[:, j, :] = Wp1[h * D2:(h + 1) * D2, :] * WP_SCALE
            t2[:, j, :] = Wp2[h * D:(h + 1) * D, :] * WP_SCALE
        shared[f"wp1dr{i}"] = t1.astype(f8)
        shared[f"wp2dr{i}"] = t2.astype(f8)
    in_maps = []
    for b in range(N_CORES):
        m = dict(shared)
        m["xT"] = np.ascontiguousarray(x[b].T).astype(bf)
        m["yT"] = np.ascontiguousarray(y[b].T).astype(bf)
        in_maps.append(m)
    return in_maps


def kernel(**inputs):
    runner = _get_runner()
    in_maps = _prep_in_maps(inputs)
    results = runner(in_maps)
    out = np.stack([results[b]["outT"].T for b in range(N_CORES)], axis=0)
    return out.astype(np.float32)


if __name__ == "__main__":
    rng = np.random.default_rng(0)
    s = 0.02
    inputs = {
        "x": rng.standard_normal((8, NT, 84), dtype=np.float32),
        "y": rng.standard_normal((8, NT, 50), dtype=np.float32),
        "W1": rng.standard_normal((84, C), dtype=np.float32) * s,
        "W2": rng.standard_normal((50, C), dtype=np.float32) * s,
        "Wqkv1": rng.standard_normal((C, 1536), dtype=np.float32) * s,
        "Wqkv2": rng.standard_normal((C, 1536), dtype=np.float32) * s,
        "Wq1": rng.standard_normal((D, D2), dtype=np.float32) * s,
        "Wq2": rng.standard_normal((D, D2), dtype=np.float32) * s,
        "Wk2": rng.standard_normal((D, D2), dtype=np.float32) * s,
        "Wp1": rng.standard_normal((1024, C), dtype=np.float32) * s,
        "bp1": np.zeros(C, np.float32),
        "Wp2": rng.standard_normal((C, C), dtype=np.float32) * s,
        "bp2": np.zeros(C, np.float32),
    }
    out = kernel(**inputs)
    print("out", out.shape, out.dtype, np.abs(out).max())


# revision 17
# speedup vs baseline: 1.0309x; 1.0309x over previous
"""nn_CrossAttention kernel for 8 Trainium2 NeuronCores.

Sharding: data-parallel over batch B=8, one batch element per core, no
collectives. Per-core layout keeps activations transposed ([feature,
token]); weights load as the stationary matmul operand.

Attention uses keys-on-partition S^T tiles (bf16 matmuls) so the PV
contraction runs over keys. exp() outputs fp8e4m3 pair-tiles
([128, 2, NT]) feeding DoubleRow fp8 matmuls: PV and the softmax
denominator (ones-row) contract 256 keys at 0.5 cycles/row — 4x the
bf16 rate. V is pre-scaled by 4 and the normalized per-head outputs by
16 (stored fp8) to stay in fp8e4m3's normal range; output-projection
weights are stored as 8x-scaled fp8 DoubleRow head-pair tiles and the
512x net scale is divided out in the final scalar_tensor_tensor.

The per-head projection work (phase C) is software-pipelined into the
attention loop so the Activation engine's exp stream (the critical
~133us) is never starved while the PE fills its slack with projections.
"""
import sys

sys.path.insert(0, "/opt/trn_rl_repo")

import numpy as np
import ml_dtypes

import concourse.bass as bass
import concourse.tile as tile
from concourse import bacc, mybir, bass2jax

F32 = mybir.dt.float32
BF16 = mybir.dt.bfloat16
F8 = mybir.dt.float8e4
EXP = mybir.ActivationFunctionType.Exp
COPY = mybir.ActivationFunctionType.Copy
DR = mybir.MatmulPerfMode.DoubleRow
ADD = mybir.AluOpType.add
MULT = mybir.AluOpType.mult

N_CORES = 8
H, D = 8, 64          # heads, head_dim
D2 = 2 * D            # 128
NT = 1024             # tokens
C = 512               # model dim
KB = 8                # key blocks of 128
SCALE = D ** -0.5
V_SCALE = 4.0         # v stored x4 (folded into w*v weights)
O_SCALE = 16.0        # normalize-mul extra scale -> o stored x64 in fp8
WP_SCALE = 8.0        # wp1/wp2 stored x8 in fp8
E_DESCALE = 1.0 / (V_SCALE * O_SCALE * WP_SCALE)


def _build(nc):
    dram = {}
    def din(name, shape, dt):
        dram[name] = nc.dram_tensor(name, shape, dt, kind="ExternalInput").ap()
    din("xT", [84, NT], BF16)
    din("yT", [50, NT], BF16)
    din("W1", [84, C], BF16)
    din("W2", [50, C], BF16)
    for n in ("w1k", "w2k", "w1v", "w2v"):
        din(n, [C, 512], BF16)
    din("w1q", [C, 1024], BF16)
    din("w2q", [C, 1024], BF16)
    din("wk2", [D, D2], BF16)
    for i in range(4):
        din(f"wp1dr{i}", [D2, 2, C], F8)
        din(f"wp2dr{i}", [D, 2, C], F8)
    din("bp1", [C], F32)
    din("bp2", [C], F32)
    outT = nc.dram_tensor("outT", [2 * C, NT], F32, kind="ExternalOutput").ap()

    with tile.TileContext(nc) as tc:
        _body(tc, nc, dram, outT)
    return dram, outT


def _body(tc, nc, dram, outT):
    from contextlib import ExitStack
    ctx = ExitStack()
    with ctx:
        wts = ctx.enter_context(tc.tile_pool(name="wts", bufs=1))
        acts = ctx.enter_context(tc.tile_pool(name="acts", bufs=1))
        big = ctx.enter_context(tc.tile_pool(name="big", bufs=2, space="PSUM"))
        psO = ctx.enter_context(tc.tile_pool(name="psO", bufs=2, space="PSUM"))
        k2pool = ctx.enter_context(tc.tile_pool(name="k2pool", bufs=4))
        ptpool = ctx.enter_context(tc.tile_pool(name="ptpool", bufs=3))
        rpool = ctx.enter_context(tc.tile_pool(name="rpool", bufs=2))
        rbpool = ctx.enter_context(tc.tile_pool(name="rbpool", bufs=2))
        outp = ctx.enter_context(tc.tile_pool(name="outp", bufs=2))

        def load(pool, name, shape, dt, src_ap=None):
            t = pool.tile(shape, dt, tag=name, name=name)
            nc.sync.dma_start(out=t, in_=dram[name] if src_ap is None else src_ap)
            return t

        # ---- inputs first so phase A starts immediately ----
        xts = load(wts, "xT", [84, NT], BF16)
        yts = load(wts, "yT", [50, NT], BF16)
        w1 = load(wts, "W1", [84, C], BF16)
        w2 = load(wts, "W2", [50, C], BF16)

        # persistent activations
        xcb = [acts.tile([128, NT], BF16, tag=f"xcb{j}", name=f"xcb{j}") for j in range(4)]
        ycb = [acts.tile([128, NT], BF16, tag=f"ycb{j}", name=f"ycb{j}") for j in range(4)]
        knew = [acts.tile([D2, NT], BF16, tag=f"kn{h}", name=f"kn{h}") for h in range(H)]
        q1p = [acts.tile([D2, NT], BF16, tag=f"q1p{h}", name=f"q1p{h}") for h in range(H)]
        q2p = [acts.tile([D2, NT], BF16, tag=f"q2p{h}", name=f"q2p{h}") for h in range(H)]
        # fp8 v pair tiles: [128, H, 2(kb-slot), 144]; col 128 = ones row
        vaugp = [acts.tile([128, H, 2, 144], F8, tag=f"va{p}", name=f"va{p}") for p in range(4)]
        # fp8 DR pair tiles for output projection operands
        o1n = [acts.tile([D2, 2, NT], F8, tag=f"o1n{i}", name=f"o1n{i}") for i in range(4)]
        o2n = [acts.tile([D, 2, NT], F8, tag=f"o2n{i}", name=f"o2n{i}") for i in range(4)]
        resqb = [acts.tile([128, NT], BF16, tag=f"rq{j}", name=f"rq{j}") for j in range(8)]
        onesdr = wts.tile([128, 2, 16], F8, tag="onesdr", name="onesdr")
        nc.vector.memset(onesdr, 1.0)
        for p in range(4):
            nc.vector.memset(vaugp[p][:, :, :, 128:129], 1.0)

        # ---- phase A: xcb = (W1^T @ xT) bf16, ycb = (W2^T @ yT) ----
        for (w, src, dstb, on_act) in ((w1, xts, xcb, True), (w2, yts, ycb, False)):
            kdim = w.shape[0]
            for j in range(4):
                ps = big.tile([128, NT], F32, tag="ps", name="psA")
                for nb in range(2):
                    nc.tensor.matmul(ps[:, nb * 512:(nb + 1) * 512],
                                     w[0:kdim, j * 128:(j + 1) * 128],
                                     src[0:kdim, nb * 512:(nb + 1) * 512],
                                     start=True, stop=True)
                if on_act:
                    nc.scalar.activation(dstb[j], ps, COPY)
                else:
                    nc.vector.tensor_copy(dstb[j], ps)

        # ---- remaining weight loads (in exact use order) ----
        wk_tiles = {}
        for wk in ("w1k", "w2k"):
            wk_tiles[wk] = [load(wts, f"{wk}_{k}", [128, 512], BF16,
                                 dram[wk][k * 128:(k + 1) * 128, :]) for k in range(4)]
        wq_tiles = {"w1q": [load(wts, f"w1q_{k}", [128, 1024], BF16,
                                 dram["w1q"][k * 128:(k + 1) * 128, :])
                            for k in range(4)]}
        wv_tiles = {}
        for wv in ("w1v", "w2v"):
            wv_tiles[wv] = [load(wts, f"{wv}_{k}", [128, 512], BF16,
                                 dram[wv][k * 128:(k + 1) * 128, :]) for k in range(4)]
        wq_tiles["w2q"] = [load(wts, f"w2q_{k}", [128, 1024], BF16,
                               dram["w2q"][k * 128:(k + 1) * 128, :])
                           for k in range(4)]
        wk2 = wts.tile([D2, D2], BF16, tag="wk2", name="wk2")
        nc.sync.dma_start(out=wk2[D:D2, :], in_=dram["wk2"])
        wp1 = [load(wts, f"wp1dr{i}", [D2, 2, C], F8) for i in range(4)]
        wp2 = [load(wts, f"wp2dr{i}", [D, 2, C], F8) for i in range(4)]
        bp1 = wts.tile([128, 4], F32, tag="bp1", name="bp1")
        nc.sync.dma_start(out=bp1, in_=dram["bp1"].rearrange("(j p) -> p j", j=4))
        bp2 = wts.tile([128, 4], F32, tag="bp2", name="bp2")
        nc.sync.dma_start(out=bp2, in_=dram["bp2"].rearrange("(j p) -> p j", j=4))

        # ---- work chunk generators (each chunk ~1-2us of PE) ----
        def chunk_v(kb, side):
            # v projection for key block kb, one side -> vaugp slot
            wvt = wv_tiles["w1v" if side == 0 else "w2v"]
            srcb = xcb if side == 0 else ycb
            lo = 0 if side == 0 else D
            ps = big.tile([128, NT], F32, tag="ps", name="psB")
            for k in range(4):
                nc.tensor.matmul(ps[:, 0:512], srcb[k][:, kb * 128:(kb + 1) * 128],
                                 wvt[k], start=(k == 0), stop=(k == 3))
            dst = vaugp[kb // 2][:, :, kb % 2, lo:lo + D]
            src = ps[:, 0:512].rearrange("p (h d) -> p h d", h=H)
            if side == 0:
                nc.vector.tensor_copy(dst, src)
            else:
                nc.gpsimd.tensor_copy(dst, src)

        def chunk_k(pr, side, nbs=(0, 1)):
            # k projection for head pair (2pr, 2pr+1), one side, nb halves
            wkt = wk_tiles["w1k" if side == 0 else "w2k"]
            srcb = xcb if side == 0 else ycb
            lo = 0 if side == 0 else D
            for nb in nbs:
                sl_ = slice(nb * 512, (nb + 1) * 512)
                ps = big.tile([128, NT], F32, tag="ps", name="psCk")
                for k in range(4):
                    nc.tensor.matmul(ps[:, 0:512],
                                     wkt[k][:, pr * 128:(pr + 1) * 128],
                                     srcb[k][:, sl_],
                                     start=(k == 0), stop=(k == 3))
                eng0, eng1 = (nc.vector, nc.gpsimd) if side == 0 else (nc.gpsimd, nc.vector)
                eng0.tensor_copy(knew[2 * pr][lo:lo + D, sl_], ps[0:D, 0:512])
                eng1.tensor_copy(knew[2 * pr + 1][lo:lo + D, sl_], ps[D:128, 0:512])

        def chunk_q(h, side, nbs=(0, 1)):
            # folded q projection for head h, one side, nb halves
            wqt = wq_tiles["w1q" if side == 0 else "w2q"]
            srcb = xcb if side == 0 else ycb
            dst = q1p if side == 0 else q2p
            for nb in nbs:
                sl_ = slice(nb * 512, (nb + 1) * 512)
                ps = big.tile([128, NT], F32, tag="ps", name="psCq")
                for k in range(4):
                    nc.tensor.matmul(ps[:, 0:512],
                                     wqt[k][:, h * 128:(h + 1) * 128],
                                     srcb[k][:, sl_],
                                     start=(k == 0), stop=(k == 3))
                eng = nc.vector if side == 0 else nc.gpsimd
                eng.tensor_copy(dst[h][:, sl_], ps[:, 0:512])
            if side == 0 and nbs[-1] == 1:
                # residual + q1r + bias, precomputed for phase E
                rsrc = xcb[h] if h < 4 else ycb[h - 4]
                bias = bp1[:, h:h + 1] if h < 4 else bp2[:, h - 4:h - 3]
                nc.vector.scalar_tensor_tensor(resqb[h], rsrc, bias, q1p[h], ADD, ADD)

        k2pt = {}
        def chunk_k2p(h):
            # k2p = Wk2^T @ k2 for head h (base-64 operands)
            ps = big.tile([128, NT], F32, tag="ps", name="psK2")
            for nb in range(2):
                nc.tensor.matmul(ps[:, nb * 512:(nb + 1) * 512],
                                 wk2[D:D2, :],
                                 knew[h][D:D2, nb * 512:(nb + 1) * 512],
                                 start=True, stop=True)
            t = k2pool.tile([D2, NT], BF16, tag="k2p", name=f"k2p{h}")
            nc.vector.tensor_copy(t, ps)
            k2pt[h] = t

        def c_chunks(pr):
            yield lambda: chunk_k(pr, 0)
            yield lambda: chunk_k(pr, 1)
            yield lambda: chunk_q(2 * pr, 0)
            yield lambda: chunk_q(2 * pr + 1, 0)
            yield lambda: chunk_q(2 * pr, 1)
            yield lambda: chunk_q(2 * pr + 1, 1)
            yield lambda: chunk_k2p(2 * pr)
            yield lambda: chunk_k2p(2 * pr + 1)

        # ---- prologue: minimal work before the exp stream starts ----
        chunk_k(0, 0)
        chunk_k(0, 1)
        chunk_q(0, 0)
        for kb in range(4):
            for s in range(2):
                chunk_v(kb, s)

        # chunk fifo: one ~0.85us half-chunk per kb-pair boundary (matches
        # the PE surplus per pair so the exp stream never starves); ordered
        # so every chunk lands >= half a head before its consumer
        def halves(f, *a):
            return [lambda: f(*a, nbs=(0,)), lambda: f(*a, nbs=(1,))]
        K, Q, P2 = chunk_k, chunk_q, chunk_k2p
        fifo = (halves(Q, 1, 0) + halves(Q, 1, 1) + [lambda: P2(1)]
                + halves(Q, 2, 0) + halves(K, 1, 0) + halves(K, 1, 1)
                + halves(Q, 2, 1) + [lambda: P2(2)]
                + halves(Q, 3, 0) + halves(Q, 3, 1) + [lambda: P2(3)]
                + halves(K, 2, 0) + halves(K, 2, 1)
                + halves(Q, 4, 0) + halves(Q, 4, 1) + [lambda: P2(4)]
                + halves(Q, 5, 0) + halves(Q, 5, 1) + [lambda: P2(5)]
                + halves(K, 3, 0) + halves(K, 3, 1)
                + halves(Q, 6, 0) + halves(Q, 6, 1) + [lambda: P2(6)]
                + halves(Q, 7, 0) + halves(Q, 7, 1) + [lambda: P2(7)])

        # ---- attention: PVs deferred one pair behind the exp stream so
        # accumulator-psum WARs never block it; head 0 br1 fully defers and
        # absorbs the remaining v fills ----
        def attention_head(h):
            defer = (h == 0)
            ops1 = psO.tile([128, NT], F32, tag="psO", name=f"ops1_{h}")
            ops2 = psO.tile([128, NT], F32, tag="psO", name=f"ops2_{h}")
            for br in range(2):
                lhs = knew[h] if br == 0 else k2pt[h]
                qin = q1p[h] if br == 0 else q2p[h]
                pend = []
                for pr in range(4):
                    ptp = ptpool.tile([128, 2, NT], F8, tag="pt", name="pt")
                    pend.append((pr, ptp))
                    for sl in range(2):
                        kb = 2 * pr + sl
                        sps = big.tile([128, NT], F32, tag="ps", name="psS")
                        for nb in range(2):
                            nc.tensor.matmul(sps[:, nb * 512:(nb + 1) * 512],
                                             lhs[:, kb * 128:(kb + 1) * 128],
                                             qin[:, nb * 512:(nb + 1) * 512],
                                             start=True, stop=True)
                        nc.scalar.activation(ptp[:, sl, :], sps, EXP, scale=SCALE)
                        if defer and br == 0:
                            chunk_v(4 + kb // 2, kb % 2)  # v kb4..7
                    if defer and br == 0:
                        continue  # PVs deferred until all v ready
                    if len(pend) > 1:
                        p0, t0 = pend.pop(0)
                        self_pv(br, p0, t0, h, ops1, ops2)
                        if fifo:
                            fifo.pop(0)()
                if defer and br == 0:
                    chunk_q(0, 1)      # q2p[0]: needed at br2 start
                    chunk_k2p(0)       # k2pt[0]: needed at br2 start
                for p0, t0 in pend:
                    self_pv(br, p0, t0, h, ops1, ops2)
                if not (defer and br == 0) and fifo:
                    fifo.pop(0)()
                # normalize
                i, sl8 = h // 2, h % 2
                rr = rpool.tile([1, NT], F32, tag="rr", name="rr")
                rrb = rbpool.tile([128, NT], F32, tag="rrb", name="rrb")
                if br == 0:
                    nc.vector.reciprocal(rr, ops2[0:1, :])
                    nc.gpsimd.partition_broadcast(rrb, rr)
                    nc.vector.scalar_tensor_tensor(
                        o1n[i][:, sl8, :], ops1[:], O_SCALE, rrb, MULT, MULT)
                else:
                    nc.vector.reciprocal(rr, ops2[D:D + 1, :])
                    nc.gpsimd.partition_broadcast(rrb[0:D, :], rr)
                    nc.vector.scalar_tensor_tensor(
                        o2n[i][:, sl8, :], ops2[0:D, :], O_SCALE, rrb[0:D, :],
                        MULT, MULT)

        def self_pv(br, pr, ptp, h, ops1, ops2):
            for nb in range(2):
                sl_ = slice(nb * 512, (nb + 1) * 512)
                if br == 0:
                    nc.tensor.matmul(ops1[:, sl_], vaugp[pr][:, h, :, 0:D2],
                                     ptp[:, :, sl_], start=(pr == 0),
                                     stop=(pr == 3), perf_mode=DR)
                    nc.tensor.matmul(ops2[0:1, sl_], onesdr[:, :, 0:1],
                                     ptp[:, :, sl_], start=(pr == 0),
                                     stop=(pr == 3), perf_mode=DR)
                else:
                    nc.tensor.matmul(ops2[0:D + 1, sl_],
                                     vaugp[pr][:, h, :, D:D2 + 1],
                                     ptp[:, :, sl_], start=(pr == 0),
                                     stop=(pr == 3), perf_mode=DR,
                                     skip_group_check=True)

        for h in range(H):
            attention_head(h)
        assert not fifo, f"{len(fifo)} chunks unconsumed"

        # ---- phase E: output projections (fp8 DR) + residuals ----
        for (wp, on, rqoff, rowoff) in ((wp1, o1n, 0, 0), (wp2, o2n, 4, C)):
            for j in range(4):
                zps = big.tile([128, NT], F32, tag="ps", name="psE")
                for nb in range(2):
                    sl_ = slice(nb * 512, (nb + 1) * 512)
                    for i in range(4):
                        nc.tensor.matmul(zps[:, sl_],
                                         wp[i][:, :, j * 128:(j + 1) * 128],
                                         on[i][:, :, sl_],
                                         start=(i == 0), stop=(i == 3),
                                         perf_mode=DR)
                of = outp.tile([128, NT], F32, tag="of", name="of")
                eng = nc.vector if j % 2 == 0 else nc.gpsimd
                eng.scalar_tensor_tensor(of, zps, E_DESCALE,
                                         resqb[rqoff + j], MULT, ADD)
                nc.sync.dma_start(
                    out=outT[rowoff + j * 128:rowoff + (j + 1) * 128, :], in_=of)


class _Runner:
    def __init__(self):
        import jax
        from jax.sharding import Mesh, PartitionSpec
        from jax.experimental.shard_map import shard_map

        nc = bacc.Bacc("TRN2", target_bir_lowering=False, debug=False,
                       num_devices=N_CORES)
        _build(nc)
        nc.compile()
        self.nc = nc

        bass2jax.install_neuronx_cc_hook()
        part_name = nc.partition_id_tensor.name if nc.partition_id_tensor else None
        in_names, out_names, out_avals, self.zero_shapes = [], [], [], []
        for alloc in nc.m.functions[0].allocations:
            if not isinstance(alloc, mybir.MemoryLocationSet):
                continue
            name = alloc.memorylocations[0].name
            if alloc.kind == "ExternalInput":
                if name != part_name:
                    in_names.append(name)
            elif alloc.kind == "ExternalOutput":
                out_names.append(name)
                shape = tuple(alloc.tensor_shape)
                dtype = mybir.dt.np(alloc.dtype)
                out_avals.append(jax.core.ShapedArray(shape, dtype))
                self.zero_shapes.append((shape, dtype))
        self.in_names, self.out_names, self.out_avals = in_names, out_names, out_avals
        n_params, n_outs = len(in_names), len(out_avals)
        all_names = in_names + out_names + ([part_name] if part_name else [])

        def _bodyfn(*args):
            operands = list(args)
            if part_name:
                operands.append(bass2jax.partition_id_tensor())
            outs = bass2jax._bass_exec_p.bind(
                *operands, out_avals=tuple(out_avals), in_names=tuple(all_names),
                out_names=tuple(out_names), lowering_input_output_aliases=(),
                sim_require_finite=True, sim_require_nnan=True, nc=nc)
            return tuple(outs)

        devices = jax.devices()[:N_CORES]
        mesh = Mesh(np.asarray(devices), ("core",))
        self._fn = jax.jit(
            shard_map(_bodyfn, mesh=mesh,
                      in_specs=(PartitionSpec("core"),) * (n_params + n_outs),
                      out_specs=(PartitionSpec("core"),) * n_outs,
                      check_rep=False),
            donate_argnums=tuple(range(n_params, n_params + n_outs)),
            keep_unused=True)
        self._jax = jax

    def __call__(self, in_maps):
        concat_in = [np.concatenate([m[n] for m in in_maps], axis=0)
                     for n in self.in_names]
        zeros = [np.zeros((N_CORES * s[0], *s[1:]), d) for s, d in self.zero_shapes]
        outs = self._fn(*concat_in, *zeros)
        self._jax.block_until_ready(outs)
        return [
            {n: np.asarray(outs[i]).reshape(N_CORES, *self.out_avals[i].shape)[c]
             for i, n in enumerate(self.out_names)}
            for c in range(N_CORES)
        ]


_RUNNER = None


def _get_runner():
    global _RUNNER
    if _RUNNER is None:
        _RUNNER = _Runner()
    return _RUNNER


def _prep_in_maps(inputs):
    f32 = np.float32
    bf = ml_dtypes.bfloat16
    f8 = ml_dtypes.float8_e4m3
    x = np.asarray(inputs["x"], f32)
    y = np.asarray(inputs["y"], f32)
    Wqkv1 = np.asarray(inputs["Wqkv1"], np.float64)
    Wqkv2 = np.asarray(inputs["Wqkv2"], np.float64)
    Wq1 = np.asarray(inputs["Wq1"], np.float64)
    Wq2 = np.asarray(inputs["Wq2"], np.float64)
    w1q = np.zeros((C, 1024), np.float64)
    w2q = np.zeros((C, 1024), np.float64)
    for h in range(H):
        w1q[:, h * D2:(h + 1) * D2] = Wqkv1[:, h * D:(h + 1) * D] @ Wq1
        w2q[:, h * D2:(h + 1) * D2] = Wqkv2[:, h * D:(h + 1) * D] @ Wq2
    Wp1 = np.asarray(inputs["Wp1"], f32)
    Wp2 = np.asarray(inputs["Wp2"], f32)
    shared = {
        "W1": np.ascontiguousarray(inputs["W1"]).astype(bf),
        "W2": np.ascontiguousarray(inputs["W2"]).astype(bf),
        "w1k": Wqkv1[:, 512:1024].astype(bf),
        "w2k": Wqkv2[:, 512:1024].astype(bf),
        "w1v": (Wqkv1[:, 1024:1536] * V_SCALE).astype(bf),
        "w2v": (Wqkv2[:, 1024:1536] * V_SCALE).astype(bf),
        "w1q": w1q.astype(bf),
        "w2q": w2q.astype(bf),
        "wk2": np.asarray(inputs["Wk2"]).astype(bf),
        "bp1": np.ascontiguousarray(inputs["bp1"], f32),
        "bp2": np.ascontiguousarray(inputs["bp2"], f32),
    }
    for i in range(4):
        t1 = np.empty((D2, 2, C), f32)
        t2 = np.empty((D, 2, C), f32)
        for j in range(2):
            h = 2 * i + j
            t1[:, j, :] = Wp1[h * D2:(h + 1) * D2, :] * WP_SCALE
            t2[:, j, :] = Wp2[h * D:(h + 1) * D, :] * WP_SCALE
        shared[f"wp1dr{i}"] = t1.astype(f8)
        shared[f"wp2dr{i}"] = t2.astype(f8)
    in_maps = []
    for b in range(N_CORES):
        m = dict(shared)
        m["xT"] = np.ascontiguousarray(x[b].T).astype(bf)
        m["yT"] = np.ascontiguousarray(y[b].T).astype(bf)
        in_maps.append(m)
    return in_maps


def kernel(**inputs):
    runner = _get_runner()
    in_maps = _prep_in_maps(inputs)
    results = runner(in_maps)
    out = np.stack([results[b]["outT"].T for b in range(N_CORES)], axis=0)
    return out.astype(np.float32)


if __name__ == "__main__":
    rng = np.random.default_rng(0)
    s = 0.02
    inputs = {
        "x": rng.standard_normal((8, NT, 84), dtype=np.float32),
        "y": rng.standard_normal((8, NT, 50), dtype=np.float32),
        "W1": rng.standard_normal((84, C), dtype=np.float32) * s,
        "W2": rng.standard_normal((50, C), dtype=np.float32) * s,
        "Wqkv1": rng.standard_normal((C, 1536), dtype=np.float32) * s,
        "Wqkv2": rng.standard_normal((C, 1536), dtype=np.float32) * s,
        "Wq1": rng.standard_normal((D, D2), dtype=np.float32) * s,
        "Wq2": rng.standard_normal((D, D2), dtype=np.float32) * s,
        "Wk2": rng.standard_normal((D, D2), dtype=np.float32) * s,
        "Wp1": rng.standard_normal((1024, C), dtype=np.float32) * s,
        "bp1": np.zeros(C, np.float32),
        "Wp2": rng.standard_normal((C, C), dtype=np.float32) * s,
        "bp2": np.zeros(C, np.float32),
    }
    out = kernel(**inputs)
    print("out", out.shape, out.dtype, np.abs(out).max())
